# revision 1
# baseline (speedup 1.0000x reference)
"""Causal self-attention (B=16, T=1024, C=768, H=12) on 8 NeuronCores.

Strategy: data-parallel over batch (2 batches per core, no collectives).
Per-core pipeline (per batch, T=1024 tokens):
  1. PE-transpose x -> xT [C, T] tiles.
  2. qT/kT per head-pair via W_qkv-as-stationary matmuls (fp32r).
  3. V in natural [T, D] layout via xT-as-stationary, stored strided with a
     ones column appended per head (softmax denominator comes out of the PV
     matmul for free).
  4. Per head: S^T = K Q^T (causal blocks only), exp(scale=1/8) on ScalarE,
     diagonal-block mask on DVE, then PV with V_aug as stationary and
     exp(S^T) as moving operand -> unnormalized y^T plus denominator row.
  5. Normalize with reciprocal + PE outer-product broadcast, write y^T.
  6. Output projection with y^T tiles as stationary -> natural layout out.
"""

import os
import numpy as np
from contextlib import ExitStack

import concourse.bass as bass
import concourse.mybir as mybir
import concourse.tile as tile
from concourse.bass import ds, ts
from concourse.vector_clock import ScopedClock
from concourse.bass_utils import run_bass_kernel_spmd

F32 = mybir.dt.float32
F32R = mybir.dt.float32r  # fp32 storage, reduced-precision matmul at full PE rate

B, T, C, H = 16, 1024, 768, 12
D = C // H           # 64
NCORES = 8
B_LOC = B // NCORES  # 2
KT = C // 128        # 6 contraction tiles
TT = T // 128        # 8 token tiles
NPAIR = H // 2       # 6 head pairs (2 heads = 128 qk rows)
EXP = mybir.ActivationFunctionType.Exp


SplitDrainTC = tile.TileContext


def split_multi_waits(nc):
    """Hoist surplus sync waits onto standalone EventSemaphore instructions.

    The walrus build in this environment rejects any instruction carrying
    more than one sync wait ("Too many sync wait commands"). Engine queues
    execute in order, so waiting on each semaphore in a preceding
    EventSemaphore instruction is equivalent to waiting on all of them at
    the original instruction.
    """
    n_split = 0
    for f in nc.m.functions:
        for blk in f.blocks:
            out = []
            for inst in blk.instructions:
                si = inst.sync_info
                if si is not None and si.on_wait and len(si.on_wait) > 1:
                    waits = list(si.on_wait)
                    for w in waits[:-1]:
                        n_split += 1
                        ev = mybir.InstEventSemaphore(
                            name=f"I-waitsplit-{n_split}",
                            ins=[],
                            outs=[],
                            engine=inst.engine,
                            sync_info=mybir.SyncInfo(on_wait=[w], on_update=[]),
                        )
                        out.append(ev)
                    si.on_wait = waits[-1:]
                out.append(inst)
            blk.instructions[:] = out
    return n_split


def build_program(split_waits=True, level=None):
    """split_waits: apply the multi-wait splitting (required for neuronx-cc
    codegen, but the CoreSim race detector rejects the synthetic
    EventSemaphore instructions — pass False when simulating)."""
    if level is None:
        level = int(os.environ.get("BUILD_LEVEL", "5"))
    nc = bass.Bass()
    x = nc.declare_dram_parameter("x", [B_LOC, T, C], F32, isOutput=False)
    wqkv = nc.declare_dram_parameter("wqkv", [C, 3 * C], F32, isOutput=False)
    wproj = nc.declare_dram_parameter("wproj", [C, C], F32, isOutput=False)
    bqkt = nc.declare_dram_parameter("bqkt", [128, 2 * NPAIR], F32, isOutput=False)
    bvbc = nc.declare_dram_parameter("bvbc", [128, C], F32, isOutput=False)
    bobc = nc.declare_dram_parameter("bobc", [128, C], F32, isOutput=False)
    maskut = nc.declare_dram_parameter("maskut", [128, 128], F32, isOutput=False)
    ident = nc.declare_dram_parameter("ident", [128, 128], F32, isOutput=False)
    ones64 = nc.declare_dram_parameter("ones64", [128, D], F32, isOutput=False)
    out = nc.declare_dram_parameter("out", [B_LOC, T, C], F32, isOutput=True)

    with tile.TileContext(nc) as tc, ExitStack() as ctx, \
            nc.allow_low_precision(reason="fp32r matmul operands"):
        consts = ctx.enter_context(tc.tile_pool(name="consts", bufs=1))
        wq_pool = ctx.enter_context(tc.tile_pool(name="wq", bufs=1))
        wp_pool = ctx.enter_context(tc.tile_pool(name="wp", bufs=1))
        xstage = ctx.enter_context(tc.tile_pool(name="xstage", bufs=2))
        xt_pool = ctx.enter_context(tc.tile_pool(name="xt", bufs=1))
        qk_pool = ctx.enter_context(tc.tile_pool(name="qk", bufs=2))
        va_pool = ctx.enter_context(tc.tile_pool(name="va", bufs=1))
        pexp = ctx.enter_context(tc.tile_pool(name="pexp", bufs=3))
        yt_pool = ctx.enter_context(tc.tile_pool(name="yt", bufs=1))
        lpool = ctx.enter_context(tc.tile_pool(name="lpool", bufs=1))
        ostage = ctx.enter_context(tc.tile_pool(name="ostage", bufs=3))
        pmm = ctx.enter_context(tc.tile_pool(name="pmm", bufs=2, space="PSUM"))
        st_pool = ctx.enter_context(tc.tile_pool(name="st", bufs=2, space="PSUM"))
        ypool = ctx.enter_context(tc.tile_pool(name="ypool", bufs=1, space="PSUM"))

        ident_sb = consts.tile([128, 128], F32)
        nc.sync.dma_start(ident_sb[:], ident[:])
        mask_sb = consts.tile([128, 128], F32)
        nc.sync.dma_start(mask_sb[:], maskut[:])
        bqk_sb = consts.tile([128, 2 * NPAIR], F32)
        nc.sync.dma_start(bqk_sb[:], bqkt[:])
        bvbc_sb = consts.tile([128, C], F32)
        nc.sync.dma_start(bvbc_sb[:], bvbc[:])
        bobc_sb = consts.tile([128, C], F32)
        nc.sync.dma_start(bobc_sb[:], bobc[:])
        ones_sb = consts.tile([128, D], F32R)
        nc.sync.dma_start(ones_sb[:], ones64[:].bitcast(F32R))

        wq = []
        for k in range(KT):
            wt = wq_pool.tile([128, 3 * C], F32R, tag=f"wq{k}", name=f"wq{k}")
            nc.sync.dma_start(wt[:], wqkv[ts(k, 128), :].bitcast(F32R))
            wq.append(wt)
        wp = []
        for k in range(KT):
            wt = wp_pool.tile([128, C], F32R, tag=f"wp{k}", name=f"wp{k}")
            nc.sync.dma_start(wt[:], wproj[ts(k, 128), :].bitcast(F32R))
            wp.append(wt)

        for b in range(B_LOC):
            # ---- Phase A: x^T ------------------------------------------
            xT = []
            for k in range(KT):
                xt = xt_pool.tile([128, T], F32R, tag=f"xT{k}", name=f"xT{k}")
                xT.append(xt)
            for tt in range(TT):
                xs_t = xstage.tile([128, C], F32, tag="xstage", name="xs_t")
                nc.sync.dma_start(xs_t[:], x[b, ts(tt, 128), :])
                for k in range(KT):
                    ptr = pmm.tile([128, 128], F32, tag="mm", name="ptr")
                    nc.tensor.transpose(ptr[:], xs_t[:, ts(k, 128)], ident_sb[:])
                    nc.vector.tensor_copy(xT[k][:, ts(tt, 128)], ptr[:])

            if level <= 1:
                for k in range(KT):
                    nc.sync.dma_start(out[b, ts(k, 128), :], xT[k][:, 0:C].bitcast(F32))
                continue

            # ---- Phase C: V_aug [T, 12*(64+1)] -------------------------
            va_tiles = []
            for tt in range(TT):
                va = va_pool.tile([128, H * (D + 1)], F32R, tag=f"va{tt}", name=f"va{tt}")
                va3 = va.rearrange("p (h e) -> p h e", e=D + 1)
                nc.vector.tensor_copy(
                    va3[:, :, D : D + 1],
                    ones_sb[:, 0:H].rearrange("p (h o) -> p h o", o=1),
                )
                for half in range(2):
                    pv = pmm.tile([128, 384], F32, tag="mm", name="pv")
                    for k in range(KT):
                        nc.tensor.matmul(
                            pv[:],
                            lhsT=(xT[k][:, ts(tt, 128)]),
                            rhs=(wq[k][:, ds(2 * C + 384 * half, 384)]),
                            start=(k == 0),
                            stop=(k == KT - 1),
                        )
                    nc.vector.tensor_add(
                        va3[:, ds(6 * half, 6), 0:D],
                        pv[:].rearrange("p (h e) -> p h e", e=D),
                        bvbc_sb[:, ds(384 * half, 384)].rearrange(
                            "p (h e) -> p h e", e=D
                        ),
                    )
                va_tiles.append(va)

            if level <= 2:
                for tt in range(TT):
                    nc.sync.dma_start(out[b, ts(tt, 128), :], va_tiles[tt][:, 0:C].bitcast(F32))
                continue

            # ---- Phases B+D: per head-pair projections + attention -----
            for p in range(NPAIR):
                qT = qk_pool.tile([128, T], F32R, tag="qT", name="qT")
                kTt = qk_pool.tile([128, T], F32R, tag="kT", name="kTt")
                for dst, colbase, bcol in (
                    (qT, 128 * p, p),
                    (kTt, C + 128 * p, NPAIR + p),
                ):
                    for half in range(2):
                        pq = pmm.tile([128, 512], F32, tag="mm", name="pq")
                        for k in range(KT):
                            nc.tensor.matmul(
                                pq[:],
                                lhsT=(wq[k][:, ds(colbase, 128)]),
                                rhs=(xT[k][:, ds(512 * half, 512)]),
                                start=(k == 0),
                                stop=(k == KT - 1),
                            )
                        nc.scalar.add(
                            dst[:, ds(512 * half, 512)],
                            pq[:],
                            bqk_sb[:, ds(bcol, 1)],
                        )

                if level <= 3:
                    nc.sync.dma_start(out[b, ts(p, 128), 0:512], qT[:, 0:512].bitcast(F32))
                    nc.sync.dma_start(out[b, ts(p, 128), 512:768], kTt[:, 0:256].bitcast(F32))
                    continue

                for h2 in range(2):
                    h = 2 * p + h2
                    pb = D * h2  # partition base for this head inside the pair
                    py = ypool.tile([D + 1, T], F32, tag="y", name="py")
                    for i in range(TT):
                        # full valid tq range [128i, 1024) in one PSUM tile
                        # (spans two banks; each matmul stays within one)
                        cstart = 128 * i
                        wtot = T - cstart
                        chunks = []
                        cs = cstart
                        while cs < T:
                            w = min(512 - (cs % 512), T - cs)
                            chunks.append((cs, w))
                            cs += w
                        pst = st_pool.tile([128, wtot], F32, tag="st", name="pst")
                        # split at the tile-local bank boundary (512)
                        lc = 0
                        while lc < wtot:
                            w = min(512, wtot - lc)
                            nc.tensor.matmul(
                                pst[:, ds(lc, w)],
                                lhsT=(kTt[ds(pb, D), ts(i, 128)]),
                                rhs=(qT[ds(pb, D), ds(cstart + lc, w)]),
                                start=True,
                                stop=True,
                            )
                            lc += w
                        pe_t = pexp.tile([128, wtot], F32R, tag="pexp", name="pe_t")
                        nc.scalar.activation(pe_t[:], pst[:], EXP, scale=0.125)
                        nc.gpsimd.tensor_mul(
                            pe_t[:, 0:128], pe_t[:, 0:128], mask_sb[:]
                        )
                        for cs, w in chunks:
                            nc.tensor.matmul(
                                py[:, ds(cs, w)],
                                lhsT=(va_tiles[i][:, ds((D + 1) * h, D + 1)]),
                                rhs=(pe_t[:, ds(cs - cstart, w)]),
                                start=(i == 0),
                                stop=(i == TT - 1),
                                skip_group_check=True,
                            )
                    # softmax denominator: row D of py (partition 64, kept
                    # at the same partition to avoid cross-lane moves)
                    lrow = lpool.tile([D + 1, T], F32, tag="lrow", name="lrow")
                    nc.vector.tensor_copy(lrow[ds(D, 1), :], py[ds(D, 1), :])
                    lrec = lpool.tile([D + 1, T], F32R, tag="lrec", name="lrec")
                    nc.vector.reciprocal(lrec[ds(D, 1), :], lrow[ds(D, 1), :])
                    lb_sb = lpool.tile([D, T], F32, tag="lbsb", name="lb_sb")
                    for half in range(2):
                        plb = pmm.tile([D, 512], F32, tag="mm", name="plb")
                        nc.tensor.matmul(
                            plb[:],
                            lhsT=(ones_sb[ds(D, 1), :]),
                            rhs=(lrec[ds(D, 1), ds(512 * half, 512)]),
                            start=True,
                            stop=True,
                        )
                        nc.vector.tensor_copy(lb_sb[:, ds(512 * half, 512)], plb[:])
                    if p == 0 and h2 == 0:
                        yTt = []
                        for k in range(KT):
                            yt = yt_pool.tile([128, T], F32R, tag=f"yT{k}", name=f"yT{k}")
                            yTt.append(yt)
                    if pb == 0:
                        nc.vector.tensor_mul(
                            yTt[p][ds(pb, D), :], py[ds(0, D), :], lb_sb[:]
                        )
                    else:
                        # A partition-shifted DVE write into a tile that the
                        # PE later weight-loads crashes the exec unit; stage
                        # at base 0 and let DMA do the partition move.
                        ystg = lpool.tile([D, T], F32R, tag="ystg", name="ystg")
                        nc.vector.tensor_mul(ystg[:], py[ds(0, D), :], lb_sb[:])
                        nc.sync.dma_start(yTt[p][ds(pb, D), :], ystg[:])

            if level <= 3:
                continue
            if level <= 4:
                for k in range(KT):
                    nc.sync.dma_start(out[b, ts(k, 128), :], yTt[k][:, 0:C].bitcast(F32))
                continue

            # ---- Phase E: output projection ----------------------------
            e_mode = int(os.environ.get("E_MODE", "0"))
            for tt in range(TT):
                for half in range(2):
                    po = pmm.tile([128, 384], F32, tag="mm", name="po")
                    if e_mode == 1:
                        nc.scalar.copy(po[:], yTt[0][:, 0:384].bitcast(F32))
                    else:
                        for k in range(KT):
                            lhs_e = yTt[k][:, ts(tt, 128)] if e_mode != 3 else xT[k][:, ts(tt, 128)]
                            rhs_e = wp[k][:, ds(384 * half, 384)] if e_mode != 2 else wq[k][:, ds(384 * half, 384)]
                            nc.tensor.matmul(
                                po[:],
                                lhsT=lhs_e,
                                rhs=rhs_e,
                                start=(k == 0),
                                stop=(k == KT - 1),
                            )
                    ot = ostage.tile([128, 384], F32, tag="ostage", name="ot")
                    nc.vector.tensor_add(ot[:], po[:], bobc_sb[:, ds(384 * half, 384)])
                    nc.sync.dma_start(out[b, ts(tt, 128), ds(384 * half, 384)], ot[:])

    if split_waits:
        split_multi_waits(nc)
    return nc


def make_in_maps(x, W_qkv, b_qkv, W_proj, b_proj):
    x = np.ascontiguousarray(np.asarray(x, dtype=np.float32))
    W_qkv = np.ascontiguousarray(np.asarray(W_qkv, dtype=np.float32))
    b_qkv = np.asarray(b_qkv, dtype=np.float32)
    W_proj = np.ascontiguousarray(np.asarray(W_proj, dtype=np.float32))
    b_proj = np.asarray(b_proj, dtype=np.float32)

    bqkt = np.ascontiguousarray(b_qkv[: 2 * C].reshape(2 * NPAIR, 128).T)
    bvbc = np.ascontiguousarray(np.tile(b_qkv[2 * C :].reshape(1, C), (128, 1)))
    bobc = np.ascontiguousarray(np.tile(b_proj.reshape(1, C), (128, 1)))
    maskut = np.triu(np.ones((128, 128), dtype=np.float32))
    ident = np.eye(128, dtype=np.float32)
    ones = np.ones((128, D), dtype=np.float32)

    shared = {
        "wqkv": W_qkv,
        "wproj": W_proj,
        "bqkt": bqkt,
        "bvbc": bvbc,
        "bobc": bobc,
        "maskut": maskut,
        "ident": ident,
        "ones64": ones,
    }
    in_maps = []
    for c in range(NCORES):
        m = dict(shared)
        m["x"] = np.ascontiguousarray(x[B_LOC * c : B_LOC * (c + 1)])
        in_maps.append(m)
    return in_maps


_PROGRAM = None


def kernel(x, W_qkv, b_qkv, W_proj, b_proj):
    global _PROGRAM
    if _PROGRAM is None:
        _PROGRAM = build_program()
    in_maps = make_in_maps(x, W_qkv, b_qkv, W_proj, b_proj)
    res = run_bass_kernel_spmd(_PROGRAM, in_maps, list(range(NCORES)))
    out = np.concatenate([res.results[c]["out"] for c in range(NCORES)], axis=0)
    return out.astype(np.float32)


if __name__ == "__main__":
    nc = build_program()
    print("built ok; instructions:", sum(len(bb.instructions) for f in nc.m.functions for bb in f.blocks))



# revision 11
# speedup vs baseline: 2.1706x; 2.1706x over previous
"""Causal self-attention (B=16, T=1024, C=768, H=12) on 8 NeuronCores.

Strategy: data-parallel over batch (2 batches per core, no collectives).
All matmul operands bf16 (host-cast weights/x), f32 PSUM accumulation.

Per-core structure (2 batches x 12 heads = 24 attention "head slots"):
  - x^T tiles arrive via DMA-transpose (xbar) directly from HBM -- no PE
    transposes, no staging.
  - QKV / out-proj matmuls are split into small "filler jobs" (one PSUM
    bank each) that are woven between the attention phases of each head
    so the PE stays dense while ScalarE runs the exp stream.
  - Attention per head: S^T = K Q^T (causal blocks, contraction 64),
    exp on ScalarE (scale=1/8) -> bf16, diagonal-block mask on GpSimd,
    PV with V_aug stationary (ones column appended -> softmax denominator
    falls out of the PV matmul), reciprocal_approx_fast on the denominator
    row, PE outer-product broadcast, DVE normalize.
"""

import os
import numpy as np
from contextlib import ExitStack

import concourse.bass as bass
import concourse.mybir as mybir
import concourse.tile as tile
from concourse.bass import ds, ts, AP
from concourse.bass_utils import run_bass_kernel_spmd

F32 = mybir.dt.float32
F32R = mybir.dt.float32r
BF16 = mybir.dt.bfloat16

B, T, C, H = 16, 1024, 768, 12
D = C // H           # 64
NCORES = 8
B_LOC = B // NCORES  # 2
KT = C // 128        # 6 contraction tiles
TT = T // 128        # 8 token tiles
NPAIR = H // 2       # 6 head pairs
EXP = mybir.ActivationFunctionType.Exp


def split_multi_waits(nc):
    """Hoist surplus sync waits onto standalone EventSemaphore instructions.

    The walrus build in this environment rejects any instruction carrying
    more than one sync wait ("Too many sync wait commands"). Engine queues
    execute in order, so waiting on each semaphore in a preceding
    EventSemaphore instruction is equivalent to waiting on all of them at
    the original instruction.
    """
    n_split = 0
    for f in nc.m.functions:
        for blk in f.blocks:
            out = []
            for inst in blk.instructions:
                si = inst.sync_info
                if si is not None and si.on_wait and len(si.on_wait) > 1:
                    waits = list(si.on_wait)
                    for w in waits[:-1]:
                        n_split += 1
                        ev = mybir.InstEventSemaphore(
                            name=f"I-waitsplit-{n_split}",
                            ins=[],
                            outs=[],
                            engine=inst.engine,
                            sync_info=mybir.SyncInfo(on_wait=[w], on_update=[]),
                        )
                        out.append(ev)
                    si.on_wait = waits[-1:]
                out.append(inst)
            blk.instructions[:] = out
    return n_split


def build_program(split_waits=True, level=None):
    if level is None:
        level = int(os.environ.get("BUILD_LEVEL", "5"))
    nc = bass.Bass()
    x = nc.declare_dram_parameter("x", [B_LOC, T, C], BF16, isOutput=False)
    wqkv = nc.declare_dram_parameter("wqkv", [C, 3 * C], BF16, isOutput=False)
    wproj = nc.declare_dram_parameter("wproj", [C, C], BF16, isOutput=False)
    bqkt = nc.declare_dram_parameter("bqkt", [128, 2 * NPAIR], F32, isOutput=False)
    bvbc = nc.declare_dram_parameter("bvbc", [128, C], F32, isOutput=False)
    bobc = nc.declare_dram_parameter("bobc", [128, C], F32, isOutput=False)
    maskut = nc.declare_dram_parameter("maskut", [128, 128], BF16, isOutput=False)
    onesb = nc.declare_dram_parameter("onesb", [128, D], BF16, isOutput=False)
    out = nc.declare_dram_parameter("out", [B_LOC, T, C], F32, isOutput=True)

    with tile.TileContext(nc) as tc, ExitStack() as ctx, \
            nc.allow_low_precision(reason="bf16 matmul operands"):
        consts = ctx.enter_context(tc.tile_pool(name="consts", bufs=1))
        wq_pool = ctx.enter_context(tc.tile_pool(name="wq", bufs=1))
        wp_pool = ctx.enter_context(tc.tile_pool(name="wp", bufs=1))
        xt_pool = ctx.enter_context(tc.tile_pool(name="xt", bufs=1))
        qk_pool = ctx.enter_context(tc.tile_pool(name="qk", bufs=3))
        va_pool = ctx.enter_context(tc.tile_pool(name="va", bufs=1))
        pexp = ctx.enter_context(tc.tile_pool(name="pexp", bufs=10))
        yt_pool = ctx.enter_context(tc.tile_pool(name="yt", bufs=1))
        lpool = ctx.enter_context(tc.tile_pool(name="lpool", bufs=2))
        ostage = ctx.enter_context(tc.tile_pool(name="ostage", bufs=3))
        st_pool = ctx.enter_context(tc.tile_pool(name="st", bufs=2, space="PSUM"))
        py_pool = ctx.enter_context(tc.tile_pool(name="py", bufs=1, space="PSUM"))
        pq_pool = ctx.enter_context(tc.tile_pool(name="pq", bufs=2, space="PSUM"))
        dram = ctx.enter_context(tc.tile_pool(name="dram", bufs=1, space="DRAM"))
        scr1 = dram.tile([2 * H, T], F32, tag="scr1", name="scr1")
        scr2 = dram.tile([2 * H, T], F32, tag="scr2", name="scr2")

        mask_sb = consts.tile([128, 128], BF16)
        nc.sync.dma_start(mask_sb[:], maskut[:])
        bqk_sb = consts.tile([128, 2 * NPAIR], F32)
        nc.sync.dma_start(bqk_sb[:], bqkt[:])
        bvbc_sb = consts.tile([128, C], F32)
        nc.sync.dma_start(bvbc_sb[:], bvbc[:])
        bobc_sb = consts.tile([128, C], F32)
        nc.sync.dma_start(bobc_sb[:], bobc[:])
        ones_sb = consts.tile([128, D], BF16)
        nc.sync.dma_start(ones_sb[:], onesb[:])

        wq = []
        for k in range(KT):
            wt = wq_pool.tile([128, 3 * C], BF16, tag=f"wq{k}", name=f"wq{k}")
            nc.sync.dma_start(wt[:], wqkv[ts(k, 128), :])
            wq.append(wt)
        wp = []
        for k in range(KT):
            wt = wp_pool.tile([128, C], BF16, tag=f"wp{k}", name=f"wp{k}")
            nc.sync.dma_start(wt[:], wproj[ts(k, 128), :])
            wp.append(wt)

        # ---- x^T via DMA transpose: xtall[b][p, k*T + t] = x[b, t, 128k+p]
        xtall = []
        for b in range(B_LOC):
            xt = xt_pool.tile([128, KT * T], BF16, tag=f"xt{b}", name=f"xt{b}")
            nc.sync.dma_start_transpose(
                xt[:].rearrange("p (k t) -> p k t", t=T), x[b, :, :]
            )
            xtall.append(xt)

        def xts(b, k):
            return xtall[b][:, ds(k * T, T)]

        qk_tiles = {}   # (b, 'q'|'k') -> current pair tile
        va_tiles = {}   # (b, tt) -> va tile
        yt_tiles = {}   # (b, p) -> yT tile

        # ---------- filler jobs (one PSUM bank each) ----------
        def qk_job(p, b, dst):
            # qT/kT for head pair p of batch b: [128 rows, T], bf16
            colbase = (0 if dst == "q" else C) + 128 * p
            bcol = p if dst == "q" else NPAIR + p
            dtile = qk_pool.tile([128, T], BF16, tag=f"{dst}{b}", name=f"{dst}{b}p{p}")
            qk_tiles[(b, p, dst)] = dtile
            for half in range(2):
                pq = pq_pool.tile([128, 512], F32, tag="pq", name="pq")
                for k in range(KT):
                    nc.tensor.matmul(
                        pq[:],
                        lhsT=wq[k][:, ds(colbase, 128)],
                        rhs=xts(b, k)[:, ds(512 * half, 512)],
                        start=(k == 0),
                        stop=(k == KT - 1),
                    )
                nc.scalar.add(
                    dtile[:, ds(512 * half, 512)],
                    pq[:],
                    bqk_sb[:, ds(bcol, 1)],
                )

        def v_job(b, tt):
            # V_aug [128 tokens, 12*(64+1)] bf16 for token tile tt of batch b
            va = va_pool.tile([128, H * (D + 1)], BF16, tag=f"va{b}{tt}",
                              name=f"va{b}{tt}")
            va_tiles[(b, tt)] = va
            va3 = va.rearrange("p (h e) -> p h e", e=D + 1)
            nc.vector.tensor_copy(
                va3[:, :, D:D + 1],
                ones_sb[:, 0:H].rearrange("p (h o) -> p h o", o=1),
            )
            for chunk, (vo, w, h0, nh) in enumerate(((0, 512, 0, 8), (512, 256, 8, 4))):
                pv = pq_pool.tile([128, 512], F32, tag="pq", name="pv")
                for k in range(KT):
                    nc.tensor.matmul(
                        pv[:, 0:w],
                        lhsT=xts(b, k)[:, ts(tt, 128)],
                        rhs=wq[k][:, ds(2 * C + vo, w)],
                        start=(k == 0),
                        stop=(k == KT - 1),
                    )
                nc.vector.tensor_add(
                    va3[:, ds(h0, nh), 0:D],
                    pv[:, 0:w].rearrange("p (h e) -> p h e", e=D),
                    bvbc_sb[:, ds(vo, w)].rearrange("p (h e) -> p h e", e=D),
                )

        def op_job(b, tt):
            # out[b, tt-tile, :] = yT^T @ Wproj + b_proj
            ot = ostage.tile([128, C], F32, tag="ostage", name="ot")
            for chunk, (vo, w) in enumerate(((0, 512), (512, 256))):
                po = pq_pool.tile([128, 512], F32, tag="pq", name="po")
                for k in range(KT):
                    nc.tensor.matmul(
                        po[:, 0:w],
                        lhsT=yt_tiles[(b, k)][:, ts(tt, 128)],
                        rhs=wp[k][:, ds(vo, w)],
                        start=(k == 0),
                        stop=(k == KT - 1),
                    )
                nc.vector.tensor_add(ot[:, ds(vo, w)], po[:, 0:w],
                                     bobc_sb[:, ds(vo, w)])
            nc.sync.dma_start(out[b, ts(tt, 128), :], ot[:])

        # ---------- filler schedule ----------
        def qk_pair_jobs(p, b):
            return [lambda p=p, b=b: qk_job(p, b, "q"),
                    lambda p=p, b=b: qk_job(p, b, "k")]

        filler = []
        for p in (1, 2, 3):
            filler += qk_pair_jobs(p, 0)
        filler += [lambda: v_job(1, 0), lambda: v_job(1, 1)]
        filler += qk_pair_jobs(4, 0)
        filler += [lambda: v_job(1, 2), lambda: v_job(1, 3)]
        filler += qk_pair_jobs(5, 0)
        filler += [lambda t=t: v_job(1, t) for t in (4, 5, 6, 7)]
        filler += qk_pair_jobs(0, 1) + qk_pair_jobs(1, 1)
        assert len(filler) == 22
        # per-slot counts for the 12 b0 slots
        b0_counts = [2, 2, 2, 2, 2, 2, 2, 2, 2, 2, 1, 1]
        b1_fill = []
        b1_fill.append(qk_pair_jobs(2, 1))
        b1_fill.append([lambda: op_job(0, 0), lambda: op_job(0, 1)])
        b1_fill.append(qk_pair_jobs(3, 1))
        b1_fill.append([lambda: op_job(0, 2), lambda: op_job(0, 3)])
        b1_fill.append(qk_pair_jobs(4, 1))
        b1_fill.append([lambda: op_job(0, 4), lambda: op_job(0, 5)])
        b1_fill.append(qk_pair_jobs(5, 1))
        b1_fill.append([lambda: op_job(0, 6), lambda: op_job(0, 7)])
        b1_fill += [[] for _ in range(4)]

        # ---------- attention ----------
        def attention_head(b, p, h2, fill_jobs):
            h = 2 * p + h2
            pb = D * h2
            qb = qk_tiles[(b, p, "q")]
            kb = qk_tiles[(b, p, "k")]
            # S-block + exp stream
            pe_list = []
            for i in range(TT):
                cstart = 128 * i
                wtot = T - cstart
                st = st_pool.tile([128, T], F32, tag="st", name="st")
                lc = 0
                while lc < wtot:
                    w = min(512 - (lc % 512), wtot - lc)
                    nc.tensor.matmul(
                        st[:, ds(lc, w)],
                        lhsT=kb[ds(pb, D), ts(i, 128)],
                        rhs=qb[ds(pb, D), ds(cstart + lc, w)],
                        start=True,
                        stop=True,
                    )
                    lc += w
                pe_t = pexp.tile([128, T], BF16, tag="pexp", name="pe_t")
                nc.scalar.activation(pe_t[:, 0:wtot], st[:, 0:wtot], EXP, scale=0.125)
                nc.gpsimd.tensor_mul(pe_t[:, 0:128], pe_t[:, 0:128], mask_sb[:])
                pe_list.append(pe_t)

            # filler jobs (projection matmuls) while ScalarE chews the exps
            for job in fill_jobs:
                job()

            # PV-block: accumulate over key tiles; diagonal chunk last
            py = py_pool.tile([D + 1, T], F32, tag="py", name="py")
            for i in range(TT):
                cstart = 128 * i
                chunks = []
                cs = cstart
                while cs < T:
                    w = min(512 - (cs % 512), T - cs)
                    chunks.append((cs, w))
                    cs += w
                for cs, w in chunks[1:] + chunks[:1]:
                    nc.tensor.matmul(
                        py[:, ds(cs, w)],
                        lhsT=va_tiles[(b, i)][:, ds((D + 1) * h, D + 1)],
                        rhs=pe_list[i][:, ds(cs - cstart, w)],
                        start=(i == 0),
                        stop=(i == TT - 1),
                        skip_group_check=True,
                    )

            # normalize. The denominator row is reshaped [1,1024]->[128,8]
            # through DRAM so the plain DVE reciprocal runs partition-
            # parallel, then broadcast [64,1024] straight from DRAM with a
            # stride-0 read. py is freed by the two copies right away.
            hidx = (b * H + h)
            lden = lpool.tile([1, T], F32, tag="lden", name="lden")
            nc.vector.tensor_copy(lden[:], py[ds(D, 1), :])
            yraw = lpool.tile([D, T], BF16, tag="yraw", name="yraw")
            nc.vector.tensor_copy(yraw[:], py[ds(0, D), :])
            nc.sync.dma_start(scr1[hidx, :], lden[0:1, :])
            ldsq = lpool.tile([128, TT], F32, tag="ldsq", name="ldsq")
            nc.sync.dma_start(
                ldsq[:], scr1[hidx, :].rearrange("(p j) -> p j", j=TT)
            )
            lrsq = lpool.tile([128, TT], F32, tag="lrsq", name="lrsq")
            nc.vector.reciprocal(lrsq[:], ldsq[:])
            nc.sync.dma_start(scr2[hidx, :], lrsq[:])
            lbb = lpool.tile([D, T], F32, tag="lbb", name="lbb")
            s1 = scr2[hidx:hidx + 1, :]
            nc.sync.dma_start(
                lbb[:], AP(s1.tensor, s1.offset, [[0, D], [1, T]])
            )
            if (b, p) not in yt_tiles:
                for k in range(KT):
                    yt_tiles[(b, k)] = yt_pool.tile(
                        [128, T], BF16, tag=f"yT{b}{k}", name=f"yT{b}{k}"
                    )
            if pb == 0:
                nc.vector.tensor_mul(
                    yt_tiles[(b, p)][ds(0, D), :], yraw[:], lbb[:]
                )
            else:
                # partition-shifted DVE write into a PE-weight-loaded tile
                # crashes the exec unit; stage at base 0, DMA does the shift.
                ystg = lpool.tile([D, T], BF16, tag="ystg", name="ystg")
                nc.vector.tensor_mul(ystg[:], yraw[:], lbb[:])
                nc.sync.dma_start(yt_tiles[(b, p)][ds(pb, D), :], ystg[:])

        # ---------- prologue ----------
        qk_job(0, 0, "q")
        qk_job(0, 0, "k")
        for tt in range(TT):
            v_job(0, tt)

        # ---------- head slots ----------
        fi = 0
        for slot in range(12):
            b, p, h2 = 0, slot // 2, slot % 2
            n = b0_counts[slot]
            jobs = filler[fi:fi + n]
            fi += n
            attention_head(b, p, h2, jobs)
        assert fi == len(filler)
        for slot in range(12):
            b, p, h2 = 1, slot // 2, slot % 2
            attention_head(b, p, h2, b1_fill[slot])

        # ---------- tail: out-proj of batch 1 ----------
        for tt in range(TT):
            op_job(1, tt)

    if split_waits:
        split_multi_waits(nc)
    return nc


def make_in_maps(x, W_qkv, b_qkv, W_proj, b_proj):
    import ml_dtypes

    bf16 = ml_dtypes.bfloat16
    x = np.ascontiguousarray(np.asarray(x, dtype=np.float32)).astype(bf16)
    W_qkv = np.ascontiguousarray(np.asarray(W_qkv, dtype=np.float32)).astype(bf16)
    W_proj = np.ascontiguousarray(np.asarray(W_proj, dtype=np.float32)).astype(bf16)
    b_qkv = np.asarray(b_qkv, dtype=np.float32)
    b_proj = np.asarray(b_proj, dtype=np.float32)

    bqkt = np.ascontiguousarray(b_qkv[: 2 * C].reshape(2 * NPAIR, 128).T)
    bvbc = np.ascontiguousarray(np.tile(b_qkv[2 * C:].reshape(1, C), (128, 1)))
    bobc = np.ascontiguousarray(np.tile(b_proj.reshape(1, C), (128, 1)))
    maskut = np.triu(np.ones((128, 128), dtype=np.float32)).astype(bf16)
    onesb = np.ones((128, D), dtype=np.float32).astype(bf16)

    shared = {
        "wqkv": W_qkv,
        "wproj": W_proj,
        "bqkt": bqkt,
        "bvbc": bvbc,
        "bobc": bobc,
        "maskut": maskut,
        "onesb": onesb,
    }
    in_maps = []
    for c in range(NCORES):
        m = dict(shared)
        m["x"] = np.ascontiguousarray(x[B_LOC * c: B_LOC * (c + 1)])
        in_maps.append(m)
    return in_maps


_PROGRAM = None


def kernel(x, W_qkv, b_qkv, W_proj, b_proj):
    global _PROGRAM
    if _PROGRAM is None:
        _PROGRAM = build_program()
    in_maps = make_in_maps(x, W_qkv, b_qkv, W_proj, b_proj)
    res = run_bass_kernel_spmd(_PROGRAM, in_maps, list(range(NCORES)))
    out = np.concatenate([res.results[c]["out"] for c in range(NCORES)], axis=0)
    return out.astype(np.float32)


if __name__ == "__main__":
    nc = build_program()
    print("built ok; instructions:",
          sum(len(bb.instructions) for f in nc.m.functions for bb in f.blocks))
